# revision 61
# baseline (speedup 1.0000x reference)
"""DiT flow model forward pass on 8 Trainium2 NeuronCores.

Data-parallel over batch (8 batches/core, T=256 tokens/core), weights
replicated. Activations live transposed [D, T] on-chip, fully in bf16
(fp32 only inside PSUM accumulation and the tiny layernorm statistics),
which keeps every DVE tensor-tensor op in the packed 2x mode and halves
DMA traffic versus fp32.

The 256 tokens are split into two independent 128-token streams (batches
0-3 / 4-7). The streams share weights but nothing else, so the scheduler
can fill one stream's serial softmax/layernorm latency chains with the
other stream's matmuls, keeping the PE array continuously busy (and
therefore at its ramped clock).

Attention computes scores transposed (scT[k,q]) so the exp output feeds
the context matmul directly with no on-chip transpose; the softmax
normalization is deferred to a single per-head-pair multiply after the
context matmul, with the row sums coming from a ones-vector matmul.

All per-layer weights arrive as ONE contiguous DMA per layer; constants
are packed into two DMAs (bf16 + f32). The hidden dimension is permuted
per-head (even rotary slots first, odd second) so RoPE becomes
elementwise muls plus a contiguous 32-partition block swap; the
permutation is folded into the weights on the host.
"""

import sys

sys.path.insert(0, "/opt/trn_rl_repo")

from contextlib import ExitStack

import ml_dtypes
import numpy as np

import bass_rust
import concourse.bass as bass
import concourse.mybir as mybir
import concourse.tile as tile
from concourse.bass_utils import run_bass_kernel_spmd
from concourse.vector_clock import ScopedClock

B, S, LD, Hh, Ww = 64, 32, 16, 32, 18
D, NH, HD, FF, L = 512, 8, 64, 2048, 6
IN = LD * Hh * Ww
EPS = 1e-5
NCORES = 8
BSH = B // NCORES          # 8 batches per core
T = BSH * S                # 256 tokens per core
TS = T // 2                # 128 tokens per stream
NEG = -30000.0             # additive mask value; exp() underflows to 0

f32 = mybir.dt.float32
bf16 = mybir.dt.bfloat16
AT = mybir.ActivationFunctionType
ALU = mybir.AluOpType

# ---------------------------------------------------------------------------
# walrus in this container accepts at most ONE inline sync-wait per
# instruction; Tile can attach several. Split extras onto NoOp carriers.
# ---------------------------------------------------------------------------

def _patched_drain_and_barrier(self, tick_clock, wait_clock):
    nc = self.nc
    ticks = list(tick_clock.global_clock)
    for p, t in enumerate(ticks):
        if t > 0:
            vc = bass_rust.VectorClock([t if i == p else 0 for i in range(len(ticks))])
            nop_inst = nc.sync.nop(nofuse=True, hint=f"tailw{p}")
            wait_clock.add_sem_waits(nop_inst.ins, ScopedClock({None: vc}))
    nc.sync.drain()
    nc.all_engine_barrier()
    popped = nc._tile_sem_poison_stack.pop()
    assert popped is self._sem_poison
    nc.clear_and_free_semaphores(list(self.sems.allocated().values()))
    nc.all_engine_barrier()


def _split_multi_waits(nc, max_waits=1):
    for f in nc.m.functions:
        for blk in f.blocks:
            idx = 0
            while idx < len(blk.instructions):
                inst = blk.instructions[idx]
                si = inst.sync_info
                if si is not None and len(si.on_wait) > max_waits:
                    waits = list(si.on_wait)
                    for j, w in enumerate(waits[:-max_waits]):
                        carrier = mybir.InstNoOp(
                            name=f"{inst.name}_wsplit{j}",
                            engine=inst.engine,
                            bass_nofuse=True,
                            sync_info=mybir.SyncInfo(on_wait=[w], on_update=[]),
                        )
                        nc.register_instruction(carrier)
                        blk.instructions.insert(idx, carrier)
                        idx += 1
                    si.on_wait = waits[-max_waits:]
                idx += 1


tile.TileContext._drain_and_barrier = _patched_drain_and_barrier

# ---------------------------------------------------------------------------
# host-side helpers
# ---------------------------------------------------------------------------

def _cast(x):
    return np.ascontiguousarray(np.asarray(x, np.float32)).astype(ml_dtypes.bfloat16)


def _perm_src():
    p = np.empty(D, dtype=np.int64)
    for h in range(NH):
        for j in range(HD // 2):
            p[h * HD + j] = h * HD + 2 * j
            p[h * HD + HD // 2 + j] = h * HD + 2 * j + 1
    return p


def _kt_major(w, kt):
    """[kt*128, n] -> [128, kt*n] with per-partition kt-major layout."""
    n = w.shape[1]
    return w.reshape(kt, 128, n).transpose(1, 0, 2).reshape(128, kt * n)


# bf16 const-block column offsets (cb tile layout)
CB_CT = 0                   # [128, 4, 128] rope cos
CB_STS = 512                # [128, 4, 128] rope +-sin
CB_MASK = 1024              # [128, 128]  causal mask, lhsT (query-major)
CB_IDW = 1152               # [128, 128]  identity
CB_PSW = 1280               # [128, 128]  rope pair swap
CB_ONE = 1408               # [128, 128]  all-ones
CB_OND = 1536               # [128, 1]    1/D
CB_COLS = 1537

# f32 const-block column offsets (cf tile: [128, 4, X] view, X = CF_X)
CF_LNC = 0                  # 4*L cols: ln w/b pairs
CF_QKB = 4 * L              # 2*L cols: q/k bias
CF_OBT = 6 * L              # L cols: attn out bias
CF_FF2B = 7 * L             # L cols: ff2 bias
CF_INPB = 8 * L             # 1 col: input-proj bias
CF_X = 8 * L + 1
# separate [128, 16, L] region for ff1 bias appended after the 4-view region
CF_FF1B = 4 * CF_X
CF_COLS = 4 * CF_X + 16 * L

_CACHE = {}


def _build(nlayers, has_lnb=False):
    nc = bass.Bass()

    xT = nc.dram_tensor("xT", [IN, T], bf16, kind="ExternalInput")
    inp_wT = nc.dram_tensor("inp_wT", [IN, D], bf16, kind="ExternalInput")
    outp_wT = nc.dram_tensor("outp_wT", [D, IN], bf16, kind="ExternalInput")
    wla_d = [nc.dram_tensor(f"wla_{l}", [128, 8192], bf16, kind="ExternalInput")
             for l in range(nlayers)]
    wlb_d = [nc.dram_tensor(f"wlb_{l}", [128, 16384], bf16, kind="ExternalInput")
             for l in range(nlayers)]
    cb_d = nc.dram_tensor("cb", [128, CB_COLS], bf16, kind="ExternalInput")
    cf_d = nc.dram_tensor("cf", [128, CF_COLS], f32, kind="ExternalInput")
    bvr_d = nc.dram_tensor("bvr", [nlayers, 3 * D], bf16, kind="ExternalInput")
    out_d = nc.dram_tensor("out", [T, IN], bf16, kind="ExternalOutput")

    with tile.TileContext(nc) as tc, ExitStack() as top:
        cp = top.enter_context(tc.tile_pool(name="consts", bufs=1))
        ap = top.enter_context(tc.tile_pool(name="acts", bufs=8))
        stp = top.enter_context(tc.tile_pool(name="stats", bufs=8))

        # ---- constants (two DMAs + one tiny one) --------------------------
        cb = cp.tile([128, CB_COLS], bf16, tag="cb")
        nc.sync.dma_start(cb[:], cb_d[:])
        cf = cp.tile([128, CF_COLS], f32, tag="cf")
        nc.sync.dma_start(cf[:], cf_d[:])
        epsc = cp.tile([1, 1], f32, tag="epsc")
        nc.vector.memset(epsc[:], EPS)

        ct = cb[:, CB_CT:CB_CT + 512].rearrange("p (m t) -> p m t", m=4)
        sts = cb[:, CB_STS:CB_STS + 512].rearrange("p (m t) -> p m t", m=4)
        maskH = cb[:, CB_MASK:CB_MASK + 128]
        idw = cb[:, CB_IDW:CB_IDW + 128]
        pswap = cb[:, CB_PSW:CB_PSW + 128]
        ones = cb[:, CB_ONE:CB_ONE + 128]
        oneD = cb[:, CB_OND:CB_OND + 1]
        cf4 = cf[:, :CF_FF1B].rearrange("p (m x) -> p m x", m=4)
        ff1b = cf[:, CF_FF1B:].rearrange("p (m x) -> p m x", m=16)

        # weight pool opened before the input-projection pool so layer-0
        # weights prefetch during the input projection
        wp = top.enter_context(tc.tile_pool(name="wl", bufs=2))
        glp = top.enter_context(tc.tile_pool(name="gl", bufs=2))
        pmm = top.enter_context(tc.tile_pool(name="ps_mm", bufs=2, space="PSUM"))
        patt = top.enter_context(tc.tile_pool(name="ps_att", bufs=1, space="PSUM"))

        # ---- input projection: hT[D, T] = (x @ inp_w.T).T ------------------
        CHUNKS = [2, 2] + [4] * 17  # k-tiles per streamed chunk (sum 72)
        hT = [None, None]
        with tc.tile_pool(name="inp_sb", bufs=4) as ip:
            hps = [pmm.tile([128, T], f32, tag=f"mm{m % 2}", bufs=2, name=f"hps{m}")
                   for m in range(4)]
            k0 = 0
            for kc, CH in enumerate(CHUNKS):
                xc = ip.tile([128, CH, T], bf16, tag="xc",
                             padded_shape=[128, 4, T], name=f"xc{kc}")
                nc.sync.dma_start(
                    xc[:], xT[k0 * 128:(k0 + CH) * 128, :]
                    .rearrange("(kt p) t -> p kt t", p=128))
                wc = ip.tile([128, CH, D], bf16, tag="wc",
                             padded_shape=[128, 4, D], name=f"wc{kc}")
                nc.sync.dma_start(
                    wc[:], inp_wT[k0 * 128:(k0 + CH) * 128, :]
                    .rearrange("(kt p) n -> p kt n", p=128))
                for kk in range(CH):
                    first = k0 + kk == 0
                    last = k0 + kk == 71
                    for m in range(4):
                        nc.tensor.matmul(hps[m][:], wc[:, kk, m * 128:(m + 1) * 128],
                                         xc[:, kk, :], start=first, stop=last)
                k0 += CH
            for s in range(2):
                h_s = ap.tile([128, 4, TS], bf16, tag=f"act{s}", bufs=3,
                              name=f"hT0_{s}")
                for m in range(4):
                    nc.scalar.activation(h_s[:, m], hps[m][:, s * TS:(s + 1) * TS],
                                         AT.Identity,
                                         bias=cf4[:, m, CF_INPB:CF_INPB + 1], scale=1.0)
                hT[s] = h_s

        # ---- transformer layers: two streams software-pipelined -----------
        # Stream B's phase i is emitted OFF phase-slots after stream A's, so
        # each stream's serial softmax/layernorm chains are covered by the
        # other stream's matmul-dense phases in every in-order engine queue.
        st = [{"h": hT[0]}, {"h": hT[1]}]
        wgt = {}

        def ph_w(l):
            def f():
                wa = wp.tile([128, 8192], bf16, tag="wa", name=f"wla{l}")
                nc.sync.dma_start(wa[:], wla_d[l][:])
                wb = wp.tile([128, 16384], bf16, tag="wb", name=f"wlb{l}")
                nc.sync.dma_start(wb[:], wlb_d[l][:])
                bvl = stp.tile([1, 3 * D], bf16, tag="bv", bufs=2, name=f"bvl{l}")
                nc.sync.dma_start(bvl[:], bvr_d[l:l + 1, :])
                wgt[l] = {
                    "qkv": wa[:, 0:6144].rearrange("p (k n) -> p k n", k=4),
                    "out": wa[:, 6144:8192].rearrange("p (k n) -> p k n", k=4),

                    "ff1": wb[:, 0:8192].rearrange("p (k n) -> p k n", k=4),
                    "ff2": wb[:, 8192:16384].rearrange("p (k n) -> p k n", k=16),
                    "bv": bvl,
                }
            return f

        def ph_rope(s, l):
            def f():
                h = st[s]["h"]
                swp = pmm.tile([128, 4, TS], f32, tag=f"mm{s}", name=f"swp{s}")
                for m in range(4):
                    nc.tensor.matmul(swp[:, m], pswap, h[:, m], start=True, stop=True)
                t2 = ap.tile([128, 4, TS], bf16, tag=f"rsc{s}", bufs=2, name=f"t2_{s}")
                t1 = ap.tile([128, 4, TS], bf16, tag=f"rsc{s}", bufs=2, name=f"t1_{s}")
                nc.vector.tensor_mul(t2[:], h[:], ct)
                nc.vector.tensor_mul(t1[:], swp[:], sts)
                hr = ap.tile([128, 4, TS], bf16, tag=f"actb{s}", bufs=4, name=f"hrT{s}")
                for m in range(4):      # per-m: k-slices release progressively
                    nc.vector.tensor_add(hr[:, m], t2[:, m], t1[:, m])
                st[s]["hr"] = hr
            return f

        def ph_qk(s, l):
            def f():
                wqkv = wgt[l]["qkv"]
                hr = st[s]["hr"]
                for qk, key in ((0, "q"), (1, "k")):
                    ps = pmm.tile([128, 4, TS], f32, tag=f"mm{s}", name=f"qk{qk}_{s}")
                    # m-outer: PSUM accumulation groups must stay sequential
                    # within a bank (a group's start clears the whole bank's
                    # has_written bits)
                    for m in range(4):
                        for k in range(4):
                            nc.tensor.matmul(
                                ps[:, m],
                                wqkv[:, k, qk * D + m * 128:qk * D + (m + 1) * 128],
                                hr[:, k], start=(k == 0), stop=(k == 3))
                    dst = ap.tile([128, 4, TS], bf16, tag=f"actb{s}", bufs=4,
                                  name=f"{key}T{s}")
                    for m in range(4):
                        nc.vector.tensor_scalar_add(
                            dst[:, m], ps[:, m],
                            cf4[:, m, CF_QKB + 2 * l + qk:CF_QKB + 2 * l + qk + 1])
                    st[s][key] = dst
            return f

        def ph_v(s, l):
            def f():
                wqkv = wgt[l]["qkv"]
                h = st[s]["h"]
                ps = pmm.tile([128, D], f32, tag=f"mm{s}", name=f"vps{s}")
                for k in range(4):
                    nc.tensor.matmul(ps[:], h[:, k], wqkv[:, k, 2 * D:3 * D],
                                     start=(k == 0), stop=False)
                nc.tensor.matmul(ps[:], ones[0:1, :], wgt[l]["bv"][:, 0:D],
                                 start=False, stop=True)
                v_s = ap.tile([128, D], bf16, tag=f"v{s}", bufs=2, name=f"v{s}")
                nc.scalar.activation(v_s[:], ps[:], AT.Identity)
                st[s]["v"] = v_s
            return f

        def ph_att_a(s, l, g):
            def f():
                sc = patt.tile([128, 4, TS], f32, tag=f"sc{s}", bufs=2,
                               name=f"sc{s}_{g}")
                for j in range(2):
                    kt = 2 * g + j
                    for hh in range(2):
                        pb = 64 * hh
                        q4 = 2 * j + hh
                        nc.tensor.matmul(sc[:, q4], maskH, idw, start=True, stop=False)
                        nc.tensor.matmul(sc[:, q4], st[s]["k"][pb:pb + 64, kt],
                                         st[s]["q"][pb:pb + 64, kt],
                                         start=False, stop=True)
                expT = ap.tile([128, 4, TS], bf16, tag=f"ex{s}", bufs=2,
                               name=f"expT{s}_{g}")
                nc.scalar.activation(expT[:], sc[:], AT.Exp)
                sums = sc[0:1, :, :]
                nc.tensor.matmul(sums, ones[:, 0:1], expT[:], start=True, stop=True)
                rinv = stp.tile([1, 4, TS], bf16, tag=f"ri{s}", bufs=2,
                                name=f"rinv{s}_{g}")
                with nc.allow_low_precision(reason="softmax denom in bf16"):
                    nc.vector.reciprocal(rinv[:], sums)
                st[s][f"ex{g}"] = expT
                st[s][f"ri{g}"] = rinv
                st[s][f"sc{g}"] = sc
            return f

        def ph_att_b(s, l, g):
            def f():
                expT = st[s][f"ex{g}"]
                rinv = st[s][f"ri{g}"]
                if g == 0:
                    st[s]["ctx"] = ap.tile([128, 4, TS], bf16, tag=f"actb{s}",
                                           bufs=4, name=f"ctxT{s}")
                # rinv broadcast reuses the retired score bank (WAR after the
                # reciprocal); the context accumulators come from the mm tag
                rb = st[s][f"sc{g}"][:, 0:2]
                cps = pmm.tile([128, 2, TS], f32, tag=f"mm{s}", name=f"cps{s}_{g}")
                for j in range(2):
                    kt = 2 * g + j
                    for hh in range(2):
                        pb = 64 * hh
                        nc.tensor.matmul(rb[pb:pb + 64, j],
                                         ones[0:1, 0:64], rinv[:, 2 * j + hh],
                                         start=True, stop=True)
                        h = 2 * kt + hh
                        nc.tensor.matmul(cps[pb:pb + 64, j],
                                         st[s]["v"][:, h * HD:(h + 1) * HD],
                                         expT[:, 2 * j + hh],
                                         start=True, stop=True)
                rbb = ap.tile([128, 2, TS], bf16, tag=f"rbb{s}", bufs=2,
                              name=f"rbb{s}_{g}")
                nc.scalar.activation(rbb[:], rb, AT.Identity)
                for j in range(2):
                    nc.vector.tensor_mul(st[s]["ctx"][:, 2 * g + j],
                                         cps[:, j], rbb[:, j])
            return f

        def ph_outp(s, l):
            def f():
                wout = wgt[l]["out"]
                bvl = wgt[l]["bv"]
                ps = pmm.tile([128, 4, TS], f32, tag=f"mm{s}", name=f"ops{s}")
                for m in range(4):
                    for k in range(4):
                        nc.tensor.matmul(ps[:, m], wout[:, k, m * 128:(m + 1) * 128],
                                         st[s]["ctx"][:, k], start=(k == 0),
                                         stop=False)
                    nc.tensor.matmul(ps[:, m], bvl[:, D + m * 128:D + (m + 1) * 128],
                                     ones[0:1, 0:TS], start=False, stop=True)
                pre = ap.tile([128, 4, TS], bf16, tag=f"res{s}", bufs=2,
                              name=f"h1pre{s}")
                nc.vector.tensor_add(pre[:], ps[:], st[s]["h"][:])
                st[s]["pre"] = pre
            return f

        def ph_ln_a(s, l):
            def f():
                src = st[s]["pre"]
                stats = pmm.tile([1, 2, TS], f32, tag=f"mm{s}", name=f"stats{s}")
                mu = stats[:, 0]
                ex2 = stats[:, 1]
                sq = ap.tile([128, 4, TS], bf16, tag=f"sq{s}", bufs=2, name=f"sq{s}")
                nc.vector.tensor_mul(sq[:], src[:], src[:])
                # groups sequential: a group's start clears the whole bank
                for k in range(4):
                    nc.tensor.matmul(mu, oneD, src[:, k], start=(k == 0), stop=(k == 3))
                for k in range(4):
                    nc.tensor.matmul(ex2, oneD, sq[:, k], start=(k == 0), stop=(k == 3))
                # the mean broadcast needs no stats-chain result: emit early
                mus = stp.tile([1, TS], bf16, tag=f"ls{s}", bufs=2, name=f"mus{s}")
                nc.scalar.activation(mus[:], mu, AT.Identity)
                bc = pmm.tile([128, 2, TS], f32, tag=f"mm{s}", name=f"bc{s}")
                nc.tensor.matmul(bc[:, 1], ones[0:1, :], mus[:], start=True, stop=True)
                bcb = ap.tile([128, 2, TS], bf16, tag=f"bcb{s}", bufs=2,
                              name=f"bcb{s}")
                nc.scalar.activation(bcb[:, 1], bc[:, 1], AT.Identity)
                t0 = ap.tile([128, 4, TS], bf16, tag=f"lns{s}", bufs=2, name=f"t0{s}")
                nc.vector.tensor_tensor(t0[:], src[:],
                                        bcb[:, 1:2, :].broadcast_to([128, 4, TS]),
                                        ALU.subtract)
                mu2 = stp.tile([1, TS], f32, tag=f"ls{s}", bufs=2, name=f"mu2{s}")
                nc.scalar.activation(mu2[:], mu, AT.Square)
                var = stp.tile([1, TS], f32, tag=f"ls{s}", bufs=2, name=f"var{s}")
                nc.vector.tensor_tensor(var[:], ex2, mu2[:], ALU.subtract)
                sd = stp.tile([1, TS], f32, tag=f"ls{s}", bufs=2, name=f"sd{s}")
                nc.scalar.activation(sd[:], var[:], AT.Sqrt, bias=epsc[0:1, 0:1],
                                     scale=1.0)
                rstd = stp.tile([1, TS], bf16, tag=f"ls{s}", bufs=2, name=f"rstd{s}")
                with nc.allow_low_precision(reason="rstd in bf16"):
                    nc.vector.reciprocal(rstd[:], sd[:])
                st[s]["rstd"] = rstd
                st[s]["bc"] = bc
                st[s]["bcb"] = bcb
                st[s]["t0"] = t0
            return f

        def ph_ln_b(s, l, wb_idx, dst_key):
            def f():
                bc = st[s]["bc"]
                bcb = st[s]["bcb"]
                nc.tensor.matmul(bc[:, 0], ones[0:1, :], st[s]["rstd"][:],
                                 start=True, stop=True)
                nc.scalar.activation(bcb[:, 0], bc[:, 0], AT.Identity)
                t1 = ap.tile([128, 4, TS], bf16, tag=f"act{s}", bufs=3, name=f"t1{s}")
                nc.vector.tensor_mul(t1[:], st[s]["t0"][:],
                                     bcb[:, 0:1, :].broadcast_to([128, 4, TS]))
                if dst_key == "h1":
                    # ln1 scale/bias are folded into the ff1 weights (host);
                    # the residual path gets u = ln1_w * t1 (bias folded into
                    # the ff2 bias row)
                    st[s]["h1"] = t1
                    u = ap.tile([128, 4, TS], bf16, tag=f"lns{s}", bufs=2,
                                name=f"u{s}")
                    for m in range(4):
                        nc.vector.tensor_scalar_mul(
                            u[:, m], t1[:, m],
                            cf4[:, m, CF_LNC + wb_idx:CF_LNC + wb_idx + 1])
                    st[s]["u"] = u
                else:
                    out = ap.tile([128, 4, TS], bf16, tag=f"act{s}", bufs=3,
                                  name=f"{dst_key}{s}")
                    for m in range(4):
                        nc.scalar.activation(out[:, m], t1[:, m], AT.Identity,
                                             bias=cf4[:, m, CF_LNC + wb_idx + 1:
                                                      CF_LNC + wb_idx + 2],
                                             scale=cf4[:, m, CF_LNC + wb_idx:
                                                       CF_LNC + wb_idx + 1])
                    st[s][dst_key] = out
            return f

        def ph_ff1(s, l, ftg):
            def f():
                if ftg == 0:
                    st[s]["gl"] = glp.tile([128, 16, TS], bf16, tag=f"gl{s}",
                                           bufs=1, name=f"gl{s}")
                gl = st[s]["gl"]
                wff1 = wgt[l]["ff1"]
                ps = pmm.tile([128, 4, TS], f32, tag=f"mm{s}", name=f"f1ps{s}")
                for ft4 in range(4):
                    ft = ftg * 4 + ft4
                    for k in range(4):
                        nc.tensor.matmul(ps[:, ft4], wff1[:, k, ft * 128:(ft + 1) * 128],
                                         st[s]["h1"][:, k], start=(k == 0),
                                         stop=(k == 3))
                for ft4 in range(4):
                    ft = ftg * 4 + ft4
                    nc.scalar.activation(gl[:, ft], ps[:, ft4], AT.Gelu,
                                         bias=ff1b[:, ft, l:l + 1], scale=1.0)
            return f

        def ph_ff2(s, l):
            def f():
                wff2 = wgt[l]["ff2"]
                gl = st[s]["gl"]
                bvl = wgt[l]["bv"]
                ps = pmm.tile([128, 4, TS], f32, tag=f"mm{s}", name=f"f2ps{s}")
                for m in range(4):
                    for k in range(16):
                        nc.tensor.matmul(ps[:, m], wff2[:, k, m * 128:(m + 1) * 128],
                                         gl[:, k], start=(k == 0), stop=False)
                    nc.tensor.matmul(ps[:, m],
                                     bvl[:, 2 * D + m * 128:2 * D + (m + 1) * 128],
                                     ones[0:1, 0:TS], start=False, stop=True)
                pre = ap.tile([128, 4, TS], bf16, tag=f"res{s}", bufs=2,
                              name=f"h2pre{s}")
                nc.vector.tensor_add(pre[:], ps[:], st[s]["u"][:])
                st[s]["pre"] = pre
            return f

        def set_h(s):
            def f():
                st[s]["h"] = st[s]["h2"]
            return f

        a_ops = []
        b_ops = []
        for l in range(nlayers):
            for s, ops in ((0, a_ops), (1, b_ops)):
                if s == 0:
                    ops.append(ph_w(l))
                else:
                    ops.append(lambda: None)
                ops.extend([
                    ph_rope(s, l), ph_qk(s, l), ph_v(s, l),
                    ph_att_a(s, l, 0), ph_att_b(s, l, 0),
                    ph_att_a(s, l, 1), ph_att_b(s, l, 1),
                    ph_outp(s, l), ph_ln_a(s, l), ph_ln_b(s, l, 4 * l, "h1"),
                    ph_ff1(s, l, 0), ph_ff1(s, l, 1), ph_ff1(s, l, 2), ph_ff1(s, l, 3),
                    ph_ff2(s, l), ph_ln_a(s, l), ph_ln_b(s, l, 4 * l + 2, "h2"),
                    set_h(s),
                ])

        OFF = 8
        for i in range(max(len(a_ops), len(b_ops) + OFF)):
            if i < len(a_ops):
                a_ops[i]()
            j = i - OFF
            if 0 <= j < len(b_ops):
                b_ops[j]()

        # ---- output projection: out[T, IN] = h @ outp_w.T ------------------
        NCH = 9
        CW = IN // NCH            # 1024 columns per chunk
        with tc.tile_pool(name="op_sb", bufs=3, side="right") as op:
            osb = {}
            for ncr in range(NCH):
                wc = op.tile([128, 4, CW], bf16, tag="wco", bufs=3)
                nc.sync.dma_start(
                    wc[:], outp_wT.rearrange("(kt p) n -> p kt n", p=128)
                    [:, :, ncr * CW:(ncr + 1) * CW])
                for s in range(2):
                    if ncr % 2 == 0:
                        osb[s] = op.tile([128, 2 * CW], bf16, tag=f"osb{s}",
                                         bufs=2, name=f"osb{s}_{ncr}")
                    for nn in range(2):
                        ps = pmm.tile([128, 512], f32, tag=f"mm{s}", name=f"out{s}")
                        for k in range(4):
                            nc.tensor.matmul(ps[:], st[s]["h"][:, k],
                                             wc[:, k, nn * 512:(nn + 1) * 512],
                                             start=(k == 0), stop=(k == 3))
                        col = (ncr % 2) * CW + nn * 512
                        if (ncr + nn) % 2 == 0:
                            nc.scalar.activation(osb[s][:, col:col + 512], ps[:],
                                                 AT.Identity)
                        else:
                            nc.vector.tensor_copy(osb[s][:, col:col + 512], ps[:])
                    if ncr % 2 == 1 or ncr == NCH - 1:
                        c0 = (ncr // 2) * 2 * CW
                        w = CW if ncr == NCH - 1 and ncr % 2 == 0 else 2 * CW
                        # out-writes go through the (idle) Pool DGE queue so
                        # they never head-of-line-block the SP weight loads
                        nc.gpsimd.dma_start(
                            out_d[s * TS:(s + 1) * TS, c0:c0 + w],
                            osb[s][:, 0:w])

    _split_multi_waits(nc)
    return nc


# ---------------------------------------------------------------------------
# host wrapper
# ---------------------------------------------------------------------------

def _prepare(inputs, nlayers):
    perm = _perm_src()
    inp_w = np.asarray(inputs["inp_w"], np.float32)
    inp_b = np.asarray(inputs["inp_b"], np.float32)
    in_proj_w = np.asarray(inputs["in_proj_w"], np.float32)
    in_proj_b = np.asarray(inputs["in_proj_b"], np.float32)
    out_w = np.asarray(inputs["out_w"], np.float32)
    out_b = np.asarray(inputs["out_b"], np.float32)
    ln1_w = np.asarray(inputs["ln1_w"], np.float32)
    ln1_b = np.asarray(inputs["ln1_b"], np.float32)
    ln2_w = np.asarray(inputs["ln2_w"], np.float32)
    ln2_b = np.asarray(inputs["ln2_b"], np.float32)
    ff1_w = np.asarray(inputs["ff1_w"], np.float32)
    ff1_b = np.asarray(inputs["ff1_b"], np.float32)
    ff2_w = np.asarray(inputs["ff2_w"], np.float32)
    ff2_b = np.asarray(inputs["ff2_b"], np.float32)
    outp_w = np.asarray(inputs["outp_w"], np.float32)

    common = {}
    common["inp_wT"] = _cast(inp_w[perm, :].T)
    common["outp_wT"] = _cast(outp_w[:, perm].T)

    # permuted-space rope tables [D, TS]
    theta = 1.0 / (10000.0 ** (np.arange(0, HD, 2, dtype=np.float32) / HD))  # [32]
    ang = np.arange(S, dtype=np.float32)[:, None] * theta[None, :]           # [S, 32]
    cos_t = np.cos(ang).astype(np.float32)
    sin_t = np.sin(ang).astype(np.float32)
    ctt = np.zeros((D, TS), np.float32)
    stt = np.zeros((D, TS), np.float32)
    s_of_t = np.tile(np.arange(S), TS // S)
    for h in range(NH):
        for j in range(HD // 2):
            ctt[h * HD + j, :] = cos_t[s_of_t, j]
            ctt[h * HD + HD // 2 + j, :] = cos_t[s_of_t, j]
            stt[h * HD + j, :] = -sin_t[s_of_t, j]
            stt[h * HD + HD // 2 + j, :] = sin_t[s_of_t, j]
    swapP = np.arange(D).reshape(NH, 2, HD // 2)[:, ::-1, :].reshape(D)

    bvr = np.zeros((nlayers, 3 * D), np.float32)
    for l in range(nlayers):
        wq = in_proj_w[l, :D][perm][:, perm] / np.sqrt(HD)
        wk = in_proj_w[l, D:2 * D][perm][:, perm]
        wv = in_proj_w[l, 2 * D:][perm][:, perm]
        common[f"wla_{l}"] = _cast(np.concatenate([
            _kt_major(np.concatenate([wq.T, wk.T, wv.T], axis=1), 4),
            _kt_major(out_w[l][perm][:, perm].T, 4),
        ], axis=1))
        ff1wf = ff1_w[l] * ln1_w[l][None, :]       # ln1 scale folded in
        common[f"wlb_{l}"] = _cast(np.concatenate([
            _kt_major(ff1wf[:, perm].T, 4),
            _kt_major(ff2_w[l][perm, :].T, 16),
        ], axis=1))
        bvr[l, 0:D] = in_proj_b[l, 2 * D:][perm]
        bvr[l, D:2 * D] = out_b[l][perm]
        bvr[l, 2 * D:] = (ff2_b[l] + ln1_b[l])[perm]
    common["bvr"] = _cast(bvr)

    # f32 consts: [128, 4, CF_X] view region + [128, 16, L] ff1 bias region
    cf4 = np.zeros((128, 4, CF_X), np.float32)

    def put4(col, vec):
        cf4[:, :, col] = vec.reshape(4, 128).T

    for l in range(nlayers):
        put4(CF_LNC + 4 * l + 0, ln1_w[l][perm])
        put4(CF_LNC + 4 * l + 1, ln1_b[l][perm])
        put4(CF_LNC + 4 * l + 2, ln2_w[l][perm])
        put4(CF_LNC + 4 * l + 3, ln2_b[l][perm])
        put4(CF_QKB + 2 * l + 0, in_proj_b[l, :D][perm] / np.sqrt(HD))
        put4(CF_QKB + 2 * l + 1, in_proj_b[l, D:2 * D][perm])
        put4(CF_OBT + l, out_b[l][perm])
        put4(CF_FF2B + l, (ff2_b[l] + ln1_b[l])[perm])
    put4(CF_INPB, inp_b[perm])
    ff1bt = np.zeros((128, 16, L), np.float32)
    for l in range(nlayers):
        ff1bf = ff1_b[l] + ff1_w[l] @ ln1_b[l]     # ln1 bias folded in
        ff1bt[:, :, l] = ff1bf.reshape(16, 128).T
    common["cf"] = np.concatenate(
        [cf4.reshape(128, 4 * CF_X), ff1bt.reshape(128, 16 * L)], axis=1)

    # bf16 consts
    cbm = np.zeros((128, CB_COLS), np.float32)
    cbm[:, CB_CT:CB_CT + 512] = _kt_major(ctt, 4)
    cbm[:, CB_STS:CB_STS + 512] = _kt_major(stt, 4)

    mask = np.full((128, 128), NEG, np.float32)          # [q, k] additive
    for blk in range(4):
        for i in range(S):
            mask[blk * S + i, blk * S:blk * S + i + 1] = 0.0
    cbm[:, CB_MASK:CB_MASK + 128] = mask
    cbm[:, CB_IDW:CB_IDW + 128] = np.eye(128, dtype=np.float32)
    psw = np.zeros((128, 128), np.float32)
    for h2 in range(2):
        b0 = 64 * h2
        for j in range(32):
            psw[b0 + 32 + j, b0 + j] = 1.0      # lhsT[k, m]: out[m] sums in[k]
            psw[b0 + j, b0 + 32 + j] = 1.0
    cbm[:, CB_PSW:CB_PSW + 128] = psw
    cbm[:, CB_ONE:CB_ONE + 128] = 1.0
    cbm[:, CB_OND:CB_OND + 1] = 1.0 / D
    common["cb"] = _cast(cbm)
    return common


def kernel(**inputs):
    nlayers = _CACHE.setdefault("nlayers", L)
    x = np.asarray(inputs["x"], np.float32)
    if "bass" not in _CACHE:
        _CACHE["bass"] = _build(nlayers)
    nc = _CACHE["bass"]
    common = _prepare(inputs, nlayers)
    in_maps = []
    for c in range(NCORES):
        m = {k: v for k, v in common.items() if not k.startswith("_")}
        xc = x[c * BSH:(c + 1) * BSH].reshape(T, IN)
        m["xT"] = _cast(xc.T)
        in_maps.append(m)
    res = run_bass_kernel_spmd(nc, in_maps, core_ids=list(range(NCORES)))
    _CACHE["res"] = res
    outp_b = np.asarray(inputs["outp_b"], np.float32)
    outs = [np.asarray(res.results[c]["out"], np.float32) + outp_b[None, :]
            for c in range(NCORES)]
    full = np.concatenate(outs, axis=0).reshape(B, S, LD, Hh, Ww)
    return full.astype(np.float32)
